# revision 18
# baseline (speedup 1.0000x reference)
"""3-layer GraphSAGE (mean aggregation) on 8 Trainium2 NeuronCores.

Strategy (graph/data parallel, per sharding hint):
  - Nodes are sharded contiguously across 8 cores; within each core, nodes are
    reordered by degree (desc) so 128-node windows have homogeneous degrees.
  - Each core's node rows are split into half A (windows 0-23) and half B
    (windows 24-48).  The replicated feature table is stored as two DRAM
    tensors: tabA = concat of all cores' A-halves, tabB = all B-halves; both
    < 32768 rows so gather indices fit int16.
  - Edges are assigned to the core that owns their dst node; stream 0 edges
    have src in A, stream 1 in B.  Within (stream, window) edges are sorted by
    src table position for HBM gather locality, padded to 128-edge blocks.
  - Layer 0 gathers from the host-provided x table (bf16) - no bootstrap
    collective.  The per-core transposed slice x^T is also host-provided.
  - Per layer, per 128-dst window: dma_gather x[src] rows (128/block), build
    one-hot S blocks from dst slots with ONE batched DVE is_equal per chunk,
    accumulate agg^T = G^T @ S into PSUM on the PE.  Stream A runs first for
    every window (scaled by 1/deg into an SBUF accumulator accA); stream B then
    accumulates per window and adds accA, so the next layer's A-gathers only
    depend on the A-half AllGather.
  - Dense part: agg@W_l + h@W_r + bias (+ReLU), transposed back to rows and
    stored to the hsl slice.  After window 23 the A-half AllGather is issued
    (overlapping B-half compute); after window 48 the B-half AllGather.
  - Final layer computes log_softmax with a single batched Ln and writes f32.
"""

import sys
import numpy as np

for _p in ("/opt/trn_rl_repo", "/root/.axon_site/_ro/trn_rl_repo"):
    if _p not in sys.path:
        sys.path.append(_p)

import ml_dtypes

BF16 = ml_dtypes.bfloat16

# Problem constants (hardcoded per spec)
N = 50000
E = 800000
D_IN = 128
D_HID = 128
D_OUT = 64
N_CORES = 8
WIN = 128
CH = 32   # staging chunk size in 128-edge blocks
import os as _os
import json as _json
CFG = {"gch": 4, "scratch": 32768, "sbuild": "pairs", "shared": True,
       "sort": "dst", "ablate": [], "queues": 4,
       "gmode": "gatherant", "tdtype": "bf16"}
CFG.update(_json.loads(_os.environ.get("KCFG", "{}")))
GCH = CFG["gch"]
SCRATCH = CFG["scratch"]
WA = 24   # windows in half A  (rows [0, 3072))
NPC = 6272
W = NPC // WIN            # 49 windows
NA = WA * WIN             # 3072 rows per core in half A
NB = NPC - NA             # 3200 rows per core in half B
TABA = N_CORES * NA       # 24576 rows
TABB = N_CORES * NB       # 25600 rows


def _plan(edge_index, n=N, n_cores=N_CORES, ch=CH, gch=GCH):
    """Host-side sharding plan: per-core degree-sorted node order, A/B half
    split, per-window padded edge-block schedule, gather index arrays (int16,
    wrapped), dst-slot arrays (bf16), inverse-degree rows."""
    src = np.asarray(edge_index[0], dtype=np.int64)
    dst = np.asarray(edge_index[1], dtype=np.int64)
    npc_raw = n // n_cores
    assert npc_raw * n_cores == n
    deg = np.bincount(dst, minlength=n).astype(np.int64)

    order = []  # order[c][r] = original node id at rank r of core c
    rank = np.empty(n, np.int64)
    for c in range(n_cores):
        ids = np.arange(c * npc_raw, (c + 1) * npc_raw)
        o = ids[np.argsort(-deg[ids], kind="stable")]
        order.append(o)
        rank[o] = np.arange(npc_raw)
    core_of = np.arange(n) // npc_raw
    # position within the stream table (A: c*NA + r ; B: c*NB + r-NA)
    in_a = rank < NA
    pos = np.where(in_a, core_of * NA + rank, core_of * NB + (rank - NA))

    ec = core_of[dst]
    er = rank[dst]
    ew = er // WIN
    edl = er % WIN
    ep = pos[src]
    es = (~in_a[src]).astype(np.int64)  # stream: 0 = A half, 1 = B half

    # group edges by (core, stream, window)
    counts = np.zeros((n_cores, 2, W), np.int64)
    np.add.at(counts, (ec, es, ew), 1)
    K = np.ceil(counts / WIN).astype(np.int64).max(axis=0)  # [2, W]
    K[0] = np.maximum(K[0], 1)  # stream A always writes every window's PSUM
    base = np.zeros_like(K)
    for s in range(2):
        base[s] = np.cumsum(K[s]) - K[s]
    B = K.sum(axis=1)  # total blocks per stream
    # pad block counts so every dma_gather call covers `gch` full blocks
    Bpad = [int(-(-int(B[s]) // gch) * gch) for s in range(2)]

    # slot arrays per core per stream (edges sorted by src pos inside window)
    if CFG["sort"] in ("src", "runs"):
        sort_key = np.lexsort((ep, ew, es, ec))
    else:
        sort_key = np.lexsort((edl, ew, es, ec))
    src_sorted = ep[sort_key]
    dl_sorted = edl[sort_key]
    idx_arrs = [[] for _ in range(n_cores)]
    dst_arrs = [[] for _ in range(n_cores)]
    ptr = 0
    for c in range(n_cores):
        for s in range(2):
            slots_i = np.zeros(Bpad[s] * WIN, np.int64)
            slots_d = np.full(Bpad[s] * WIN, -1.0, np.float32)
            for w in range(W):
                cnt = counts[c, s, w]
                o = base[s, w] * WIN
                slots_i[o : o + cnt] = src_sorted[ptr : ptr + cnt]
                slots_d[o : o + cnt] = dl_sorted[ptr : ptr + cnt]
                ptr += cnt
            if CFG["sort"] == "runs":
                slots_i = slots_i.reshape(-1, 16, 8).transpose(
                    0, 2, 1).reshape(-1)
                slots_d = slots_d.reshape(-1, 16, 8).transpose(
                    0, 2, 1).reshape(-1)
            idx_arrs[c].append(slots_i)
            dst_arrs[c].append(slots_d)
    assert ptr == len(src)

    def wrap_idx(a):
        # [128, nblk*8] int16: per gather call of `gch` blocks, gch*128 idxs
        # wrapped into 16 partitions x gch*8 cols, replicated to 128 parts.
        cols = []
        for b0 in range(0, len(a), gch * WIN):
            blk = a[b0 : b0 + gch * WIN]
            cols.append(blk.reshape(-1, 16).T)
        wr = np.concatenate(cols, axis=1).astype(np.int16)
        return np.tile(wr, (8, 1))

    plan = {
        "n": n, "n_cores": n_cores, "npc_raw": npc_raw, "ch": ch, "gch": gch,
        "K": K, "base": base, "B": B, "Bpad": Bpad, "order": order,
        "deg": deg,
        "idx": [[wrap_idx(idx_arrs[c][s]) for s in range(2)]
                for c in range(n_cores)],
        "offs": [[np.ascontiguousarray(
                      idx_arrs[c][s].reshape(Bpad[s], WIN).T.astype(np.int32))
                  for s in range(2)] for c in range(n_cores)],
        "dst": [[(np.repeat(dst_arrs[c][s].reshape(Bpad[s], WIN).T, 2, axis=1)
                  if CFG["sbuild"] == "pairs" else
                  dst_arrs[c][s].reshape(Bpad[s], WIN).T).astype(BF16)
                 for s in range(2)] for c in range(n_cores)],
    }
    return plan


def _build(plan, d_in=D_IN, d_hid=D_HID, d_out=D_OUT, repeat=1):
    import concourse.bacc as bacc
    import concourse.bass as bass
    import concourse.mybir as mybir
    import concourse.tile as tile
    from contextlib import ExitStack

    dt = mybir.dt
    ch, gch = plan["ch"], plan["gch"]
    K, base, B, Bpad = plan["K"], plan["base"], plan["B"], plan["Bpad"]
    n_cores = plan["n_cores"]
    D = d_in
    assert d_in == d_hid == 128

    nc = bacc.Bacc("TRN2", target_bir_lowering=False,
                   dynamic_dma_scratch_size=SCRATCH,
                   num_swdge_queues=CFG["queues"])

    tdt = dt.bfloat16 if CFG["tdtype"] == "bf16" else dt.float8e4
    xtabA = nc.dram_tensor("xtabA", [TABA, D], tdt, kind="ExternalInput")
    xtabB = nc.dram_tensor("xtabB", [TABB, D], tdt, kind="ExternalInput")
    xT_d = nc.dram_tensor("xT", [128, NPC], dt.bfloat16, kind="ExternalInput")
    idx_d, dst_d, offs_d = [], [], []
    for s in range(2):
        if CFG["gmode"] == "indirect":
            offs_d.append(nc.dram_tensor(f"offs{s}", [128, Bpad[s]], dt.int32,
                                         kind="ExternalInput"))
        else:
            idx_d.append(nc.dram_tensor(f"idx{s}", [128, Bpad[s] * 8],
                                        dt.int16, kind="ExternalInput"))
        dmul = 2 if CFG["sbuild"] == "pairs" else 1
        dst_d.append(nc.dram_tensor(f"dst{s}", [128, Bpad[s] * dmul],
                                    dt.bfloat16, kind="ExternalInput"))
    invdeg_d = nc.dram_tensor("invdeg", [128, NPC], dt.float32,
                              kind="ExternalInput")
    iota_d = nc.dram_tensor("iota", [128, 128], dt.bfloat16, kind="ExternalInput")
    idbf_d = nc.dram_tensor("idbf", [128, 128], dt.bfloat16, kind="ExternalInput")
    wl_d, wr_d, b_d = [], [], []
    for li, (din, dout) in enumerate([(D, d_hid), (d_hid, d_hid), (d_hid, d_out)]):
        wl_d.append(nc.dram_tensor(f"wl{li}", [din, dout], dt.bfloat16,
                                   kind="ExternalInput"))
        wr_d.append(nc.dram_tensor(f"wr{li}", [din, dout], dt.bfloat16,
                                   kind="ExternalInput"))
        b_d.append(nc.dram_tensor(f"b{li}", [128, 1], dt.float32,
                                  kind="ExternalInput"))

    hslA = [nc.dram_tensor(f"hsl{i}A", [NA, D], tdt) for i in range(2)]
    hslB = [nc.dram_tensor(f"hsl{i}B", [NB, D], tdt) for i in range(2)]
    _aspace = {"addr_space": "Shared"} if CFG["shared"] else {}
    hfA = [nc.dram_tensor(f"hf{i}A", [TABA, D], tdt, **_aspace)
           for i in range(2)]
    hfB = [nc.dram_tensor(f"hf{i}B", [TABB, D], tdt, **_aspace)
           for i in range(2)]
    out_d = nc.dram_tensor("out", [NPC, d_out], dt.float32, kind="ExternalOutput")

    groups = [list(range(n_cores))]

    with tile.TileContext(nc) as tc, ExitStack() as ctx:
        per = ctx.enter_context(tc.tile_pool(name="persist", bufs=1))
        gpool = ctx.enter_context(tc.tile_pool(name="g", bufs=2))
        spool = ctx.enter_context(tc.tile_pool(name="s", bufs=2))
        mpool = ctx.enter_context(tc.tile_pool(name="m", bufs=3))
        pp = ctx.enter_context(tc.tile_pool(name="ps_agg", bufs=2, space="PSUM"))
        pp_d = ctx.enter_context(tc.tile_pool(name="ps_dense", bufs=2, space="PSUM"))
        pp_t = ctx.enter_context(tc.tile_pool(name="ps_tr", bufs=2, space="PSUM"))

        def load_const(dram, shape, dtp, tag):
            t = per.tile(shape, dtp, tag=tag, name=tag)
            nc.sync.dma_start(out=t[:], in_=dram[:])
            return t

        iota = load_const(iota_d, [128, 128], dt.bfloat16, "iota")
        idbf = load_const(idbf_d, [128, 128], dt.bfloat16, "idbf")
        invdeg = load_const(invdeg_d, [128, NPC], dt.float32, "invdeg")
        wl = [load_const(wl_d[i], list(wl_d[i].shape), dt.bfloat16, f"wl{i}")
              for i in range(3)]
        wr = [load_const(wr_d[i], list(wr_d[i].shape), dt.bfloat16, f"wr{i}")
              for i in range(3)]
        bias = [load_const(b_d[i], [128, 1], dt.float32, f"b{i}") for i in range(3)]

        hT = [per.tile([128, NPC], dt.bfloat16, tag=f"hT{i}", name=f"hT{i}")
              for i in range(2)]
        dummyG = per.tile([128, 128],
                          dt.bfloat16 if CFG["tdtype"] == "bf16" else dt.float8e4,
                          tag="dummyG")
        if "gather" in CFG["ablate"]:
            nc.gpsimd.memset(dummyG[:], 0.0)
        accA = per.tile([128, NPC], dt.bfloat16, tag="accA", name="accA")
        outT = per.tile([128, NPC], dt.float32, tag="outT")
        preT = per.tile([128, W * d_out], dt.float32, tag="preT")
        sums = per.tile([128, W], dt.float32, tag="sums")
        lns = per.tile([128, W], dt.float32, tag="lns")
        nc.gpsimd.memset(outT[:], 0.0)

        def _body():
            # load x^T for layer-0 root path
            nc.sync.dma_start(out=hT[0][:], in_=xT_d[:])

            def stream_blocks(li, s, tab, w_range, on_block, on_window):
                """Iterate windows of one stream; stage chunks of ch blocks
                (idx load, gather in gch-block calls, dst load, one batched
                S build), then per block call on_block(w, psum, block args)."""
                cur_ck = -1
                G_t = S_t = None
                for w in w_range:
                    nb = int(K[s][w])
                    ps = None
                    for b in range(nb):
                        gb = int(base[s][w]) + b
                        ck, off = divmod(gb, ch)
                        if ck != cur_ck:
                            cur_ck = ck
                            nblk = min(ch, Bpad[s] - ck * ch)
                            if CFG["gmode"] == "indirect":
                                offs_sb = mpool.tile([128, ch], dt.int32,
                                                     tag=f"offs{s}")
                                nc.sync.dma_start(
                                    out=offs_sb[:, :nblk],
                                    in_=offs_d[s][:, ck * ch: ck * ch + nblk])
                            else:
                                idx_sb = mpool.tile([128, ch * 8], dt.int16,
                                                    tag=f"idx{s}")
                                nc.sync.dma_start(
                                    out=idx_sb[:, :nblk * 8],
                                    in_=idx_d[s][:, ck * ch * 8: ck * ch * 8 + nblk * 8])
                            if "gather" not in CFG["ablate"]:
                                G_t = gpool.tile([128, ch, 128], tdt,
                                                 tag=f"G{s}", name=f"G{s}")
                                for g0 in range(0, nblk, gch):
                                    gn = min(gch, nblk - g0)
                                    if CFG["gmode"] == "indirect":
                                        nc.gpsimd.indirect_dma_start(
                                            out=G_t[:, g0:g0 + gn, :],
                                            out_offset=None,
                                            in_=tab,
                                            in_offset=bass.IndirectOffsetOnAxis(
                                                ap=offs_sb[:, g0:g0 + gn],
                                                axis=0))
                                    else:
                                        nc.gpsimd.dma_gather(
                                            G_t[:, g0:g0 + gn, :], tab,
                                            idx_sb[:, g0 * 8:(g0 + gn) * 8],
                                            gn * 128, gn * 128, 128,
                                            queue_num=(ck * ((ch + gch - 1) // gch)
                                                       + g0 // gch) % CFG["queues"])
                            dmul = 2 if CFG["sbuild"] == "pairs" else 1
                            dst_sb = mpool.tile([128, ch * dmul], dt.bfloat16,
                                                tag=f"d{s}")
                            nc.sync.dma_start(
                                out=dst_sb[:, :nblk * dmul],
                                in_=dst_d[s][:, ck * ch * dmul:
                                             (ck * ch + nblk) * dmul])
                            S_t = spool.tile([128, ch * 128], dt.bfloat16,
                                             tag=f"S{s}", name=f"S{s}")
                            if "sbuild" in CFG["ablate"]:
                                if ck == 0:
                                    nc.vector.memset(S_t[:], 0.0)
                            elif CFG["sbuild"] == "pairs":
                                nc.vector.tensor_tensor(
                                    out=S_t[:].rearrange(
                                        "p (b r k) -> p b r k", b=ch, k=2
                                    )[:, :nblk, :, :],
                                    in0=dst_sb[:, :nblk * 2].rearrange(
                                        "p (b k) -> p b k", k=2).unsqueeze(2)
                                        .to_broadcast([128, nblk, 64, 2]),
                                    in1=iota[:].rearrange(
                                        "p (r k) -> p r k", k=2).unsqueeze(1)
                                        .to_broadcast([128, nblk, 64, 2]),
                                    op=mybir.AluOpType.is_equal)
                            elif CFG["sbuild"] == "batched":
                                nc.vector.tensor_tensor(
                                    out=S_t[:].rearrange("p (b k) -> p b k", b=ch)[:, :nblk, :],
                                    in0=dst_sb[:, :nblk].to_broadcast([128, nblk, 128]),
                                    in1=iota[:].unsqueeze(1).to_broadcast([128, nblk, 128]),
                                    op=mybir.AluOpType.is_equal)
                            else:
                                for bb in range(nblk):
                                    nc.vector.tensor_tensor(
                                        out=S_t[:, bb * 128:(bb + 1) * 128],
                                        in0=dst_sb[:, bb:bb + 1].to_broadcast([128, 128]),
                                        in1=iota[:],
                                        op=mybir.AluOpType.is_equal)
                        if ps is None:
                            ps = pp.tile([128, 128], dt.float32, tag="agg")
                        lhs_ap = (dummyG[:] if "gather" in CFG["ablate"]
                                  else G_t[:, off, :])
                        if "aggmm" not in CFG["ablate"] or b == 0:
                            nc.tensor.matmul(
                                out=ps[:], lhsT=lhs_ap,
                                rhs=S_t[:, off * 128:(off + 1) * 128],
                                start=(b == 0), stop=(b == nb - 1 or
                                                      "aggmm" in CFG["ablate"]))
                    on_window(w, ps)

            def do_layer(li, tabA, tabB, hT_in, hT_out, slA, slB, fA, fB, last):
                dout = d_out if last else d_hid

                # phase A: accumulate stream-A blocks, scale by 1/deg -> accA
                def winA(w, ps):
                    win = slice(w * 128, (w + 1) * 128)
                    nc.vector.tensor_tensor(out=accA[:, win], in0=ps[:],
                                            in1=invdeg[:, win],
                                            op=mybir.AluOpType.mult)

                stream_blocks(li, 0, tabA, range(W), None, winA)

                # phase B: stream-B blocks + accA -> dense -> store
                def winB(w, ps):
                    win = slice(w * 128, (w + 1) * 128)
                    if ps is not None:
                        tmp = mpool.tile([128, 128], dt.bfloat16, tag="tmp")
                        nc.vector.tensor_tensor(out=tmp[:], in0=ps[:],
                                                in1=invdeg[:, win],
                                                op=mybir.AluOpType.mult)
                        aggsc = mpool.tile([128, 128], dt.bfloat16, tag="aggsc")
                        nc.vector.tensor_tensor(out=aggsc[:], in0=tmp[:],
                                                in1=accA[:, win],
                                                op=mybir.AluOpType.add)
                        agg_ap = aggsc[:]
                    else:
                        agg_ap = accA[:, win]
                    pd = pp_d.tile([128, 128], dt.float32, tag="dense")
                    nc.tensor.matmul(out=pd[:dout, :], lhsT=wl[li][:], rhs=agg_ap,
                                     start=True, stop=False)
                    nc.tensor.matmul(out=pd[:dout, :], lhsT=wr[li][:],
                                     rhs=hT_in[:, win], start=False, stop=True)
                    if not last:
                        nc.scalar.activation(out=hT_out[:, win], in_=pd[:],
                                             func=mybir.ActivationFunctionType.Relu,
                                             bias=bias[li][:, :1])
                        tp = pp_t.tile([128, 128], dt.bfloat16, tag="tp")
                        nc.tensor.transpose(out=tp[:], in_=hT_out[:, win],
                                            identity=idbf[:])
                        rows = mpool.tile([128, D],
                                          dt.bfloat16 if CFG["tdtype"] == "bf16"
                                          else dt.float8e4, tag="rows_out")
                        nc.vector.tensor_copy(out=rows[:], in_=tp[:])
                        if w < WA:
                            nc.sync.dma_start(out=slA[w * 128:(w + 1) * 128, :],
                                              in_=rows[:])
                        else:
                            w2 = w - WA
                            nc.sync.dma_start(out=slB[w2 * 128:(w2 + 1) * 128, :],
                                              in_=rows[:])
                        if w == WA - 1 and "ag" not in CFG["ablate"]:
                            nc.gpsimd.collective_compute(
                                "AllGather", mybir.AluOpType.bypass,
                                replica_groups=groups,
                                ins=[slA.ap().opt()], outs=[fA.ap().opt()])
                        if w == W - 1 and "ag" not in CFG["ablate"]:
                            nc.gpsimd.collective_compute(
                                "AllGather", mybir.AluOpType.bypass,
                                replica_groups=groups,
                                ins=[slB.ap().opt()], outs=[fB.ap().opt()])
                    else:
                        nc.scalar.activation(out=outT[:dout, win],
                                             in_=pd[:dout, :],
                                             func=mybir.ActivationFunctionType.Identity,
                                             bias=bias[li][:dout, :1])

                stream_blocks(li, 1, tabB, range(W), None, winB)

            do_layer(0, xtabA[:, :], xtabB[:, :], hT[0], hT[1],
                     hslA[0], hslB[0], hfA[0], hfB[0], last=False)
            do_layer(1, hfA[0][:, :], hfB[0][:, :], hT[1], hT[0],
                     hslA[1], hslB[1], hfA[1], hfB[1], last=False)
            do_layer(2, hfA[1][:, :], hfB[1][:, :], hT[0], None,
                     None, None, None, None, last=True)

            # final: transpose outT to rows, batched log_softmax, f32 output
            idf = per.tile([128, 128], dt.float32, tag="idf")
            nc.vector.tensor_copy(out=idf[:], in_=idbf[:])
            for w in range(W):
                win = slice(w * 128, (w + 1) * 128)
                owin = slice(w * d_out, (w + 1) * d_out)
                tp = pp_t.tile([128, 128], dt.float32, tag="tpf")
                nc.tensor.transpose(out=tp[:], in_=outT[:, win], identity=idf[:])
                negmax = mpool.tile([128, 1], dt.float32, tag="negmax")
                nc.vector.tensor_reduce(out=negmax[:], in_=tp[:, :d_out],
                                        axis=mybir.AxisListType.X,
                                        op=mybir.AluOpType.max, negate=True)
                nc.vector.tensor_tensor(
                    out=preT[:, owin], in0=tp[:, :d_out],
                    in1=negmax[:, :1].to_broadcast([128, d_out]),
                    op=mybir.AluOpType.add)
                esb = mpool.tile([128, d_out], dt.float32, tag="esb")
                nc.scalar.activation(out=esb[:], in_=preT[:, owin],
                                     func=mybir.ActivationFunctionType.Exp)
                nc.vector.tensor_reduce(out=sums[:, w:w + 1], in_=esb[:],
                                        axis=mybir.AxisListType.X,
                                        op=mybir.AluOpType.add)
            nc.scalar.activation(out=lns[:], in_=sums[:],
                                 func=mybir.ActivationFunctionType.Ln)
            for w in range(W):
                win = slice(w * 128, (w + 1) * 128)
                owin = slice(w * d_out, (w + 1) * d_out)
                res = mpool.tile([128, d_out], dt.float32, tag="res")
                nc.vector.tensor_tensor(
                    out=res[:], in0=preT[:, owin],
                    in1=lns[:, w:w + 1].to_broadcast([128, d_out]),
                    op=mybir.AluOpType.subtract)
                nc.sync.dma_start(out=out_d[win, :], in_=res[:])

        for _rep in range(repeat):
            _body()

    nc.compile()
    return nc


def _make_inputs(plan, x, W1_l, W1_r, b1, Wm_l, Wm_r, bm, W2_l, W2_r, b2,
                 d_out=D_OUT):
    n_cores = plan["n_cores"]
    npc_raw = plan["npc_raw"]
    iota = np.tile(np.arange(128, dtype=np.float32)[None, :], (128, 1)).astype(BF16)
    ident = np.eye(128, dtype=np.float32).astype(BF16)

    def pad_bias(b):
        v = np.zeros((128, 1), np.float32)
        v[: len(b), 0] = np.asarray(b, np.float32)
        return v

    x = np.asarray(x, np.float32)
    TDT = BF16 if CFG["tdtype"] == "bf16" else ml_dtypes.float8_e4m3
    xb = x.astype(TDT)
    # replicated stream tables (A halves then B halves, same layout as hf)
    tabA = np.zeros((TABA, D_IN), TDT)
    tabB = np.zeros((TABB, D_IN), TDT)
    for c in range(n_cores):
        o = plan["order"][c]
        tabA[c * NA:(c + 1) * NA] = xb[o[:NA]]
        tabB[c * NB: c * NB + (npc_raw - NA)] = xb[o[NA:]]

    common = {
        "iota": iota, "idbf": ident,
        "xtabA": tabA, "xtabB": tabB,
        "wl0": np.asarray(W1_l, np.float32).astype(BF16),
        "wr0": np.asarray(W1_r, np.float32).astype(BF16),
        "b0": pad_bias(b1),
        "wl1": np.asarray(Wm_l, np.float32).astype(BF16),
        "wr1": np.asarray(Wm_r, np.float32).astype(BF16),
        "b1": pad_bias(bm),
        "wl2": np.asarray(W2_l, np.float32).astype(BF16),
        "wr2": np.asarray(W2_r, np.float32).astype(BF16),
        "b2": pad_bias(b2),
    }
    deg = plan["deg"]
    in_maps = []
    for c in range(n_cores):
        o = plan["order"][c]
        xs = np.zeros((NPC, D_IN), BF16)
        xs[:npc_raw] = x[o].astype(BF16)
        m = dict(common)
        m["xT"] = np.ascontiguousarray(xs.T)
        if CFG["gmode"] == "indirect":
            for s in range(2):
                m[f"offs{s}"] = plan["offs"][c][s]
        dg = np.concatenate([deg[o], np.ones(NPC - npc_raw, np.int64)])
        m["invdeg"] = np.tile(
            (1.0 / np.maximum(dg, 1)).astype(np.float32)[None, :], (128, 1))
        for s in range(2):
            if CFG["gmode"] != "indirect":
                m[f"idx{s}"] = plan["idx"][c][s]
            m[f"dst{s}"] = plan["dst"][c][s]
        in_maps.append(m)
    return in_maps


def _postprocess(plan, results, d_out=D_OUT):
    n = plan["n"]
    out = np.empty((n, d_out), np.float32)
    for c in range(plan["n_cores"]):
        out[plan["order"][c]] = results[c]["out"][: plan["npc_raw"]]
    return out


_CACHE = {}


def kernel(x, edge_index, W1_l, W1_r, b1, Wm_l, Wm_r, bm, W2_l, W2_r, b2,
           _trace=False):
    from concourse.bass_utils import run_bass_kernel_spmd

    edge_index = np.asarray(edge_index)
    key = hash(edge_index.tobytes())
    if key not in _CACHE:
        plan = _plan(edge_index)
        nc = _build(plan)
        _CACHE[key] = (plan, nc)
    plan, nc = _CACHE[key]
    in_maps = _make_inputs(plan, x, W1_l, W1_r, b1, Wm_l, Wm_r, bm,
                           W2_l, W2_r, b2)
    res = run_bass_kernel_spmd(nc, in_maps, core_ids=list(range(N_CORES)),
                               trace=_trace)
    out = _postprocess(plan, res.results)
    if _trace:
        kernel._last_exec_ns = res.exec_time_ns
        kernel._last_res = res
    return out
